# revision 34
# baseline (speedup 1.0000x reference)
"""Trainium2 Bass kernel for nn_DocREModel (doc-level relation extraction graph pooling).

Structure exploited: every use of `attention` reduces over heads first
(S = sum_h A[h]), and only a few rows/cols of S are ever read:
  - mention contexts need the 128 mention rows of S (full 1024 width),
  - link-span pooling only touches S[span_k x span_k] per span (<=31x31
    blocks), NOT a full union-rowset cross product.
The host ships, per doc, only gathered/quantized slices of attention; the
device does the O(big) float work: the 12-head sum, the S-row contractions
against seq, and the span pooling contractions.

Sharding: 2 cores per doc (B=4 -> 8 cores):
  - mention path split by S-columns: core g of doc b handles columns
    [512g, 512g+512); host sums the two partial numerator outputs.
  - link path split by spans: core g handles spans [8g, 8g+8) entirely;
    host concatenates the two v outputs.

All per-core inputs ship in ONE u8 blob [128, 11564] so every DMA chunk has
>=2.3KB contiguous per partition (descriptor-overhead-free). Per-partition:
  [0, 6144)      mention attention, 5-bit (x31) in u8, [(h12, rq4, m128)]
                 with p = c%128, c = 512g + 128rq + p
  [6144, 9224)   seq_aug fp8e3 [rq4, 770] (col 768 = ones for row sums)
  [9224, 10764)  seqg fp8e3 [half2, 770]: rows s_k+c at p=(k4, c32)
  [10764, 11532) span att fp8e3 [(ch3, half2, k4*32+c)], p = (h%4)*32+r
  [11532, 11564) onehot fp16 [half2, j8] = (j==4*half+k4) & (c < len_k)

Device compute:
  - 12-head sum: 5-bit values let pair-adds run on u16-packed views with no
    carry across bytes (sums stay < 256 per byte through two tree levels),
    so DVE runs at 2x with giant fused adds: two [128,3,256]-u16 adds (L1),
    one [128,3,256] (L2), one [128,256] (L3), one final u8->f16 unpack add.
  - span col-sums u[k,c] = sum_{h,r} A: 6 tiny PE matmuls against ones.
  - w^T[(k,c), j] = u * onehot via DVE per-partition tensor_scalar_mul.
  - v[j, :] = sum_{(k,c)} w^T . seqg  (PSUM-accumulated matmuls).
  - mention numerators mnum[m, :] = sum_c S[pos_m, c] seq[c, :], scaled 1/8
    into f16 outputs (row-sum in col 768).
  - PE HAM warmup: ~8 dummy matmuls at t0 so real matmuls run at 2.4 GHz.
Host combine applies the tiny normalizations (epsilon divides, entity
pooling, logsumexp, type concat) on the small results.
"""

import os
import sys

for _p in ("/opt/trn_rl_repo", "/root/.axon_site/_ro/trn_rl_repo"):
    if os.path.isdir(_p) and _p not in sys.path:
        sys.path.insert(0, _p)

import numpy as np

B, L, H, NH = 4, 1024, 768, 12
E, MPE, K = 32, 4, 16
EM = E * MPE              # 128 mentions per doc
KH = K // 2               # spans per core
TYPE_DIM = 20
OFFSET = 1
HA = H + 2                # 768 + row-sum ones col + even pad
N1 = 512                  # PSUM bank split of the 770-wide outputs
QSM = 31.0                # 5-bit quantization scale for mention attention
OSC = 1.0 / 8.0           # output scale keeping mention numerators in f16 range

# blob byte offsets (per partition)
O_ATT = 0
O_SEQ = 6144
O_SEQG = 9224
O_ASP = 10764
O_OH = 11532
O_END = 11564


def _build_nc(debug=False):
    import concourse.bass as bass
    import concourse.mybir as mybir
    import concourse.tile as tile
    from concourse import bacc

    f32 = mybir.dt.float32
    f16 = mybir.dt.float16
    u8 = mybir.dt.uint8
    u16 = mybir.dt.uint16
    f8 = mybir.dt.float8e3

    nc = bacc.Bacc("TRN2", target_bir_lowering=False, debug=debug)

    blob = nc.dram_tensor("blob", [128, O_END], u8, kind="ExternalInput")
    out_m = nc.dram_tensor("out_m", [EM, HA], f16, kind="ExternalOutput")
    out_v = nc.dram_tensor("out_v", [KH, HA], f16, kind="ExternalOutput")

    with tile.TileContext(nc) as tc:
        with (
            tc.tile_pool(name="data", bufs=1) as datap,
            tc.tile_pool(name="work", bufs=1) as workp,
            tc.tile_pool(name="ps", bufs=8, space="PSUM") as psp,
        ):
            blob_t = datap.tile([128, O_END], u8, tag="blob", name="blob")
            P6 = workp.tile([128, 6, 256], u16, tag="p6", name="p6")
            QQ = workp.tile([128, 3, 256], u16, tag="qq", name="qq")
            RR = workp.tile([128, 256], u16, tag="rr", name="rr")
            S_t = workp.tile([128, 512], f16, tag="s", name="s")
            wT = workp.tile([128, 2, KH], f16, tag="wT", name="wT")
            v_t = workp.tile([KH, HA], f16, tag="v", name="v")
            m_t = workp.tile([EM, HA], f16, tag="mout", name="mout")
            ones_t = workp.tile([128, 1], f8, tag="ones", name="ones")

            # typed views into the blob
            att16 = blob_t[:, O_ATT:O_SEQ].bitcast(u16).rearrange(
                "p (a b s) -> p a b s", a=6, b=2, s=256)      # [pair, head-in-pair]
            att8 = blob_t[:, O_ATT:O_SEQ].rearrange(
                "p (h s) -> p h s", h=NH)                     # per-head u8 view
            seq_ap = blob_t[:, O_SEQ:O_SEQG].bitcast(f8).rearrange(
                "p (r c) -> p r c", r=4)
            seqg_ap = blob_t[:, O_SEQG:O_ASP].bitcast(f8).rearrange(
                "p (t c) -> p t c", t=2)
            asp_ap = blob_t[:, O_ASP:O_OH].bitcast(f8).rearrange(
                "p (a t k) -> p a t k", a=3, t=2)
            oh_ap = blob_t[:, O_OH:O_END].bitcast(f16).rearrange(
                "p (t k) -> p t k", t=2)

            nc.vector.memset(ones_t[:], 1.0)

            # ---- input DMAs, three rings: att halves on the two HWDGE
            #      rings, seq on the gpsimd SWDGE ring (2-partition
            #      coalesced descriptors), span inputs second on sync ----
            nc.sync.dma_start(out=blob_t[:, 0:3072], in_=blob[:, 0:3072])
            nc.scalar.dma_start(out=blob_t[:, 3072:6144], in_=blob[:, 3072:6144])
            nc.gpsimd.dma_start(out=blob_t[:, O_SEQ:O_SEQG], in_=blob[:, O_SEQ:O_SEQG])
            nc.sync.dma_start(out=blob_t[:, O_SEQG:O_END], in_=blob[:, O_SEQG:O_END])

            # ---- 12-head sum tree on u16-packed views (carry-free by 5-bit
            #      quantization): L1 6 pairs in 2 ops, L2 3 pairs in 1 op,
            #      L3 1 op, final unpack add u8+u8 -> f16. ----
            p6v = P6[:].rearrange("p (a b) s -> p a b s", a=3, b=2)
            nc.vector.tensor_add(P6[:, 0:3, :], att16[:, 0:3, 0, :],
                                 att16[:, 0:3, 1, :])
            nc.vector.tensor_add(P6[:, 3:6, :], att16[:, 3:6, 0, :],
                                 att16[:, 3:6, 1, :])
            nc.vector.tensor_add(QQ[:], p6v[:, :, 0, :], p6v[:, :, 1, :])
            nc.vector.tensor_add(RR[:], QQ[:, 0, :], QQ[:, 1, :])
            nc.vector.tensor_add(S_t[:], RR[:].bitcast(u8), QQ[:, 2, :].bitcast(u8))

            # ---- span col-sums u[(k,c)] = sum_{h,r} A  (PE vs ones) ----
            ps_u0 = psp.tile([128, 1], f32, tag="ps", name="ps_u0")
            ps_u1 = psp.tile([128, 1], f32, tag="ps", name="ps_u1")
            for half, psu in ((0, ps_u0), (1, ps_u1)):
                for ch in range(3):
                    nc.tensor.matmul(psu[:], asp_ap[:, ch, half, :], ones_t[:],
                                     start=(ch == 0), stop=(ch == 2))
            nc.vector.tensor_scalar_mul(wT[:, 0, :], oh_ap[:, 0, :], ps_u0[:])
            nc.vector.tensor_scalar_mul(wT[:, 1, :], oh_ap[:, 1, :], ps_u1[:])

            # ---- link numerators v[j, :] = sum_{(k,c)} w^T[(k,c), j] seqg ----
            ps_v0 = psp.tile([KH, N1], f32, tag="ps", name="ps_v0")
            ps_v1 = psp.tile([KH, HA - N1], f32, tag="ps", name="ps_v1")
            for half in (0, 1):
                nc.tensor.matmul(ps_v0[:], wT[:, half, :], seqg_ap[:, half, 0:N1],
                                 start=(half == 0), stop=(half == 1))
                nc.tensor.matmul(ps_v1[:], wT[:, half, :], seqg_ap[:, half, N1:HA],
                                 start=(half == 0), stop=(half == 1))
            nc.scalar.copy(out=v_t[:, 0:N1], in_=ps_v0[:])
            nc.scalar.copy(out=v_t[:, N1:HA], in_=ps_v1[:])
            nc.scalar.dma_start(out=out_v[:], in_=v_t[:])

            # ---- mention numerators; 258-half first so its copy + store
            #      overlap the 512-half matmuls ----
            ps_m1 = psp.tile([EM, HA - N1], f32, tag="ps", name="ps_m1")
            ps_m0 = psp.tile([EM, N1], f32, tag="ps", name="ps_m0")
            for rc in range(4):
                nc.tensor.matmul(ps_m1[:], S_t[:, 128 * rc:128 * (rc + 1)],
                                 seq_ap[:, rc, N1:HA], start=(rc == 0), stop=(rc == 3))
            nc.scalar.mul(m_t[:, N1:HA], ps_m1[:], OSC)
            nc.sync.dma_start(out=out_m[:, N1:HA], in_=m_t[:, N1:HA])
            for rc in range(4):
                nc.tensor.matmul(ps_m0[:], S_t[:, 128 * rc:128 * (rc + 1)],
                                 seq_ap[:, rc, 0:N1], start=(rc == 0), stop=(rc == 3))
            nc.scalar.mul(m_t[:, 0:N1], ps_m0[:], OSC)
            nc.sync.dma_start(out=out_m[:, 0:N1], in_=m_t[:, 0:N1])

    nc.compile()
    return nc


_NC_CACHE = {}


def _get_nc():
    if "nc" not in _NC_CACHE:
        _NC_CACHE["nc"] = _build_nc()
    return _NC_CACHE["nc"]


def _per_core_inputs(sequence_output, attention, mention_pos, link_start, link_len):
    """Host prep: index gathers, layout transposes, quantize. No reductions."""
    import ml_dtypes
    f8 = ml_dtypes.float8_e3m4
    seq = np.asarray(sequence_output, dtype=np.float32)
    att = np.asarray(attention, dtype=np.float32)
    mpos = np.asarray(mention_pos).astype(np.int64)
    lstart = np.asarray(link_start).astype(np.int64)
    llen = np.asarray(link_len).astype(np.int64)

    in_maps = []
    metas = []
    for b in range(B):
        pos = (mpos[b] + OFFSET).reshape(EM)
        s = lstart[b] + OFFSET
        e = s + llen[b] + 1                                   # span len 1..31
        att_doc = att[b]                                      # [12, 1024, 1024]
        seq_aug = np.concatenate(
            [np.clip(seq[b], -15.0, 15.0), np.ones((L, 1), np.float32),
             np.zeros((L, 1), np.float32)], axis=1)           # [1024, 770]
        amq = np.clip(np.round(att_doc[:, pos, :] * QSM), 0, 31).astype(np.uint8)

        for g in range(2):
            blob = np.zeros((128, O_END), np.uint8)
            a = amq[:, :, g * 512:(g + 1) * 512]              # [12, 128m, 512c]
            x = a.reshape(NH, 128, 4, 128).transpose(3, 0, 2, 1)   # [p, h, rq, m]
            blob[:, O_ATT:O_SEQ] = x.reshape(128, NH * 512)

            A8 = np.zeros((KH, NH, 32, 32), np.float32)
            G = np.zeros((KH, 32, HA), np.float32)
            ohm = np.zeros((KH, 32, KH), np.float32)
            for j in range(KH):
                k = KH * g + j
                ln = int(e[k] - s[k])
                A8[j, :, :ln, :ln] = att_doc[:, s[k]:e[k], s[k]:e[k]]
                G[j] = seq_aug[s[k]:s[k] + 32]
                ohm[j, :ln, j] = 1.0
            # seq [p, (rq, 770)] fp8
            sq = seq_aug[g * 512:(g + 1) * 512].reshape(4, 128, HA).transpose(1, 0, 2)
            blob[:, O_SEQ:O_SEQG] = np.ascontiguousarray(
                sq.astype(f8)).reshape(128, 4 * HA).view(np.uint8)
            # seqg [p=(k4,c), (half, 770)] fp8
            Gp = G.reshape(2, 4, 32, HA).transpose(1, 2, 0, 3)
            blob[:, O_SEQG:O_ASP] = np.ascontiguousarray(
                Gp.astype(f8)).reshape(128, 2 * HA).view(np.uint8)
            # att_sp [p=(h%4)*32+r, (ch, half, k4*32+c)] fp8
            T = A8.astype(f8).transpose(1, 2, 0, 3)           # [h, r, k, c]
            T = T.reshape(3, 4, 32, 2, 4, 32).transpose(1, 2, 0, 3, 4, 5)
            blob[:, O_ASP:O_OH] = np.ascontiguousarray(T).reshape(128, 768).view(np.uint8)
            # onehot [p=(k4,c), (half, 8)] fp16
            Op = ohm.reshape(2, 4, 32, KH).transpose(1, 2, 0, 3)
            blob[:, O_OH:O_END] = np.ascontiguousarray(
                Op.astype(np.float16)).reshape(128, 2 * KH).view(np.uint8)

            in_maps.append({"blob": blob})
        metas.append((pos, (e - s).astype(np.float32)))
    return in_maps, metas


def _combine(outs, metas, sequence_output, type_table):
    seq = np.asarray(sequence_output, dtype=np.float32)
    ttab = np.asarray(type_table, dtype=np.float32)
    type_ids = np.concatenate(
        [np.zeros(E, np.int64), np.ones(EM, np.int64), np.full(K, 2, np.int64)])
    nodes_type = ttab[type_ids]                               # [176, 20]

    out = np.zeros((B, E + EM + K + E + EM, H + TYPE_DIM), np.float32)
    for b in range(B):
        pos, length = metas[b]
        o0, o1 = outs[2 * b], outs[2 * b + 1]
        mnum = o0["out_m"].astype(np.float32) + o1["out_m"].astype(np.float32)
        v = np.concatenate([o0["out_v"], o1["out_v"]],
                           axis=0).astype(np.float32)         # [16, 770] raw

        m_ctx = mnum[:, :H] / (mnum[:, H:H + 1] + QSM * NH * 1e-5 * OSC)
        enum = mnum.reshape(E, MPE, HA).sum(axis=1)
        e_ctx = enum[:, :H] / (enum[:, H:H + 1] + QSM * NH * MPE * 1e-5 * OSC)
        link_rep = v[:, :H] / (NH * length[:, None])

        memb = seq[b][pos]                                    # [128, 768] exact
        mg = memb.reshape(E, MPE, H)
        mmax = mg.max(axis=1)
        eemb = np.log(np.exp(mg - mmax[:, None, :]).sum(axis=1)) + mmax

        nodes_raw = np.concatenate([eemb, memb, link_rep], axis=0)    # [176, H]
        nodes = np.concatenate([nodes_raw, nodes_type], axis=1)       # [176, H+20]
        ctx = np.concatenate([e_ctx, m_ctx], axis=0)                  # [160, H]
        ctx = np.concatenate([ctx, np.zeros((E + EM, TYPE_DIM), np.float32)], axis=1)
        out[b] = np.concatenate([nodes, ctx], axis=0)
    return out


def kernel(**inputs):
    from concourse.bass_utils import run_bass_kernel_spmd

    in_maps, metas = _per_core_inputs(
        inputs["sequence_output"], inputs["attention"],
        inputs["mention_pos"], inputs["link_start"], inputs["link_len"])
    nc = _get_nc()
    res = run_bass_kernel_spmd(nc, in_maps, core_ids=list(range(8)))
    return _combine(res.results, metas, inputs["sequence_output"], inputs["type_table"])
